# revision 1
# baseline (speedup 1.0000x reference)
"""Trainium2 Bass kernel for GAT+GCN+MLP message passing (8 NeuronCores, SPMD).

Strategy (dst-node sharding), v2:
  - Host: add self-loops, pack the 10000 nodes into 8 cores x 10 tiles x 128
    slots balancing in-edge counts; build per-edge one-hot dst masks (plus
    transposed and GCN-norm-weighted variants) and edge-expanded x operands.
  - GAT (phase B): per edge-chunk, e-values via matmuls accumulated in spare
    PSUM columns; leaky(0.2) via Prelu on the scalar engine; exp written
    straight into the message-rhs tile; messages (x_src * ex, 10 heads via
    broadcast APs) aggregated per dst tile with one-hot mask matmuls into
    PSUM together with the softmax denominators; normalization via one
    broadcast DVE multiply; per-head W transform via transposes; bias +
    leaky(0.01) fused into a fixed-slope Lrelu activation readout.
  - AllGather of the hidden h (768-wide bf16) split into QT quarters; the
    first quarters are issued mid-phase-B so the collective overlaps GAT
    compute and only the final 1-tile quarter is exposed.
  - GCN+MLP (phase C): h rows fetched with batched gpsimd dma_gather calls
    (int16 wrapped indices, half-tile granularity to fit the SWDGE ring);
    aggregation with norm-weighted mask matmuls; dense stack feature-major
    with weights stationary; every dense readout is a single Lrelu
    activation with the bias applied on the partition axis. Phase-C weights
    stream in mid-phase-B on the ACT DMA queue.
"""

import os
import sys
import heapq
import dataclasses

for _p in ("/opt/trn_rl_repo", "/root/.axon_site/_ro/trn_rl_repo"):
    if os.path.isdir(_p) and _p not in sys.path:
        sys.path.insert(0, _p)

import numpy as np
import ml_dtypes

import concourse.bass as bass
import concourse.tile as tile
from concourse import bacc, mybir
from concourse.bass_utils import run_bass_kernel_spmd
from concourse.library_config import mlp as mlp_lib

BF16 = ml_dtypes.bfloat16

N = 10000
F_IN = 66
HEADS = 10
F_HEAD = 66
F_GAT = HEADS * F_HEAD          # 660
GCN_OUT = 1320
NCORE = 8
TILES_PER_CORE = 10
NTILE = NCORE * TILES_PER_CORE  # 80
NSLOT = NTILE * 128             # 10240
SLOTS_PER_CORE = TILES_PER_CORE * 128  # 1280

F32 = mybir.dt.float32
BF = mybir.dt.bfloat16
I32 = mybir.dt.int32
I16 = mybir.dt.int16
HW_H = 768   # h row width for dma_gather (256B-multiple); cols 660:768 zero
QT = [3, 3, 3, 1]   # tiles per AllGather quarter (split collective)

_CACHE = {}
SPLIT_AG = os.environ.get("KERNEL_SPLIT_AG", "1") == "1"


# ---------------------------------------------------------------- host prep

def _prep(x, edge_index):
    src = np.concatenate([edge_index[0], np.arange(N, dtype=np.int64)])
    dst = np.concatenate([edge_index[1], np.arange(N, dtype=np.int64)])
    deg = np.bincount(dst, minlength=N).astype(np.int64)

    # pack nodes into 80 tiles of <=128 slots, balancing in-edge counts
    order = np.argsort(-deg, kind="stable")
    tile_cnt = np.zeros(NTILE, np.int64)
    slot = np.empty(N, np.int64)
    hp = [(0, t) for t in range(NTILE)]
    heapq.heapify(hp)
    for n_ in order:
        while True:
            e, t = heapq.heappop(hp)
            if tile_cnt[t] < 128:
                break
        slot[n_] = t * 128 + tile_cnt[t]
        tile_cnt[t] += 1
        heapq.heappush(hp, (e + int(deg[n_]), t))
    sslot = slot[src]
    dslot = slot[dst]
    dtile = dslot >> 7
    dlocal = dslot & 127

    tile_edges = np.bincount(dtile, minlength=NTILE)
    nc_t = int(np.max((tile_edges + 127) // 128))
    e_tile = nc_t * 128
    nchunks = TILES_PER_CORE * nc_t

    esrc = np.zeros((NCORE, TILES_PER_CORE, e_tile), np.int64)
    edstl = np.full((NCORE, TILES_PER_CORE, e_tile), -1, np.int64)
    edst = np.zeros((NCORE, TILES_PER_CORE, e_tile), np.int64)
    ord_t = np.argsort(dtile, kind="stable")
    bounds = np.searchsorted(dtile[ord_t], np.arange(NTILE + 1))
    for t in range(NTILE):
        idx = ord_t[bounds[t]:bounds[t + 1]]
        k = len(idx)
        c, tt = divmod(t, TILES_PER_CORE)
        esrc[c, tt, :k] = sslot[idx]
        edstl[c, tt, :k] = dlocal[idx]
        edst[c, tt, :k] = dslot[idx]

    # one-hot masks [core][128 dst-part, nchunks*128 edges] (+ transposed)
    onehot = (edstl[..., None] == np.arange(128))      # [C,T,e_tile,128] bool
    oh = onehot.reshape(NCORE, TILES_PER_CORE, nc_t, 128, 128)
    masks = np.ascontiguousarray(
        oh.transpose(0, 3, 1, 2, 4)).reshape(
        NCORE, 128, nchunks * 128).astype(BF16)
    masksT = np.ascontiguousarray(
        oh.transpose(0, 4, 1, 2, 3)).reshape(
        NCORE, 128, nchunks * 128).astype(BF16)

    # norm-weighted masks for GCN: w = dinv[src]*dinv[dst] folded in
    dinv_slot = np.ones(NSLOT, np.float32)
    dinv_slot[slot] = 1.0 / np.sqrt(np.maximum(deg, 1).astype(np.float32))
    wvals = (dinv_slot[esrc] * dinv_slot[edst]).astype(np.float32)
    wm = oh.astype(np.float32) * wvals.reshape(
        NCORE, TILES_PER_CORE, nc_t, 128)[..., None]
    wmasks = np.ascontiguousarray(
        wm.transpose(0, 3, 1, 2, 4)).reshape(
        NCORE, 128, nchunks * 128).astype(BF16)

    # edge-expanded x operands
    x_pad = np.zeros((NSLOT, F_IN), np.float32)
    x_pad[slot] = x
    xg = np.empty((NCORE, 128, nchunks * F_IN), BF16)
    xgT = np.empty((NCORE, F_IN, nchunks * 128), BF16)
    for c in range(NCORE):
        arr = x_pad[esrc[c].reshape(-1)]               # [nidx, 66] f32
        a3 = arr.reshape(nchunks, 128, F_IN)
        xg[c] = np.ascontiguousarray(
            a3.transpose(1, 0, 2)).reshape(128, nchunks * F_IN).astype(BF16)
        xgT[c] = np.ascontiguousarray(arr.T).astype(BF16)

    # gather-row remap for the split AllGather: h_full row order is
    # [all cores' Q1 tiles | Q2 | Q3 | Q4] with quarters of QT tiles each
    sc = np.arange(NSLOT) // SLOTS_PER_CORE
    loc = np.arange(NSLOT) % SLOTS_PER_CORE
    row_of_slot = np.zeros(NSLOT, np.int64)
    lo = 0
    base = 0
    for qt in QT:
        rows = qt * 128
        m = (loc >= lo) & (loc < lo + rows)
        row_of_slot[m] = base + sc[m] * rows + (loc[m] - lo)
        lo += rows
        base += NCORE * rows
    # dma_gather index layout: per (tile, half-tile) block, flat order
    # i = chunk*128 + partition, wrapped as idx16[i % 16, i // 16] and
    # replicated down all 128 partitions. Half-tile granularity keeps each
    # gather under the 1024-descriptor SWDGE ring.
    nit = nc_t * 128                       # idxs per tile
    h1 = (nc_t + 1) // 2                   # chunks in first half
    idx16 = np.empty((NCORE, 128, TILES_PER_CORE * (nit // 16)), np.int16)
    for c in range(NCORE):
        rows = row_of_slot[esrc[c]]        # [T, nc_t*128] (chunk-major flat)
        for t in range(TILES_PER_CORE):
            o = t * (nit // 16)
            for (a, b) in ((0, h1 * 128), (h1 * 128, nit)):
                flat = rows[t][a:b]
                n = b - a
                w16 = np.zeros((16, n // 16), np.int16)
                w16[np.arange(n) % 16, np.arange(n) // 16] = flat
                idx16[c, :, o + a // 16:o + b // 16] = np.tile(w16, (8, 1))

    xT = np.zeros((F_IN, NSLOT), np.float32)
    xT[:, slot] = x.T
    xT_my = np.stack([np.ascontiguousarray(
        xT[:, c * SLOTS_PER_CORE:(c + 1) * SLOTS_PER_CORE])
        for c in range(NCORE)])

    return dict(slot=slot, nc_t=nc_t, masks=masks, masksT=masksT,
                wmasks=wmasks, xg=xg, xgT=xgT, idx16=idx16, xT_my=xT_my)


def _prep_weights(W_gat, att_src, att_dst, b_gat, W_gcn, b_gcn,
                  W_g1, b_g1, W_g2, b_g2, W_fc1, b_fc1, W_fc2, b_fc2,
                  W_out, b_out):
    Wg = np.asarray(W_gat, np.float32).reshape(F_IN, HEADS, F_HEAD)
    w_as = np.einsum("fhg,hg->fh", Wg, np.asarray(att_src, np.float32))
    w_ad = np.einsum("fhg,hg->fh", Wg, np.asarray(att_dst, np.float32))
    w_ad = np.ascontiguousarray(w_ad.astype(np.float32))          # [66,10]
    w_as_bf = np.ascontiguousarray(w_as.astype(BF16))             # [66,10]

    def chunk_pack(W, kchunks, ncols):
        W = np.asarray(W, np.float32)
        K, M = W.shape
        out = np.zeros((128, kchunks * ncols), BF16)
        for kt in range(kchunks):
            r0 = kt * 128
            r1 = min(K, r0 + 128)
            if r0 >= K:
                break
            out[:r1 - r0, kt * ncols:kt * ncols + M] = W[r0:r1].astype(BF16)
        return out

    W_gcn_p = chunk_pack(W_gcn, 6, GCN_OUT)
    W_g1_p = chunk_pack(W_g1, 11, 1000)
    W_g2_p = chunk_pack(W_g2, 8, 64)

    def col_pack(b, nch):
        b = np.asarray(b, np.float32).reshape(-1)
        out = np.zeros((nch, 128), np.float32)
        out.reshape(-1)[:b.shape[0]] = b
        return np.ascontiguousarray(out.T)

    b_gcn_cols = col_pack(b_gcn, 11)                  # [128, 11]
    b_g1_cols = col_pack(b_g1, 8)                     # [128, 8]
    W_fc1_p = np.asarray(W_fc1, BF16)
    W_fc2_p = np.asarray(W_fc2, BF16)
    W_out_p = np.asarray(W_out, BF16)
    b_tail = np.zeros((128, 4), np.float32)
    b_tail[:64, 0] = np.asarray(b_g2, np.float32)
    b_tail[:32, 1] = np.asarray(b_fc1, np.float32)
    b_tail[:16, 2] = np.asarray(b_fc2, np.float32)
    b_tail[0, 3] = float(np.asarray(b_out).reshape(-1)[0])

    ident = np.eye(128, dtype=BF16)
    ones_row = np.ones((1, 128), BF16)
    b_gat_row = np.zeros((1, F_GAT), BF16)
    b_gat_row[0, :] = np.asarray(b_gat, BF16)
    W_heads = np.asarray(W_gat, BF16)

    return dict(w_ad=w_ad, w_as_bf=w_as_bf, W_gcn_p=W_gcn_p, W_g1_p=W_g1_p,
                W_g2_p=W_g2_p, b_gcn_cols=b_gcn_cols, b_g1_cols=b_g1_cols,
                W_fc1_p=W_fc1_p, W_fc2_p=W_fc2_p, W_out_p=W_out_p,
                b_tail=b_tail, ident=ident, b_gat_row=b_gat_row,
                ones_row=ones_row, W_heads=W_heads)


# ---------------------------------------------------------------- device kernel

def _bc(ap, pattern):
    """Replace the free dims of a (sliced) AP with explicit [step,count] dims."""
    return dataclasses.replace(
        ap, ap=[list(ap.ap[0])] + [list(p) for p in pattern])


def _build(nc_t, repeat=1, phases="ABGC", dump_h=False):
    nchunks = TILES_PER_CORE * nc_t

    nc = bacc.Bacc("TRN2", target_bir_lowering=False, debug=False,
                   num_devices=NCORE, num_swdge_queues=1)

    def inp(name, shape, dt):
        return nc.dram_tensor(name, list(shape), dt, kind="ExternalInput")

    xg_d = inp("xg", [128, nchunks * F_IN], BF)
    xgT_d = inp("xgT", [F_IN, nchunks * 128], BF)
    masks_d = inp("masks", [128, nchunks * 128], BF)
    masksT_d = inp("masksT", [128, nchunks * 128], BF)
    wmasks_d = inp("wmasks", [128, nchunks * 128], BF)
    idx16_d = inp("idx16", [128, TILES_PER_CORE * nc_t * 8], I16)
    xTmy_d = inp("xT_my", [F_IN, SLOTS_PER_CORE], F32)
    w_ad_d = inp("w_ad", [F_IN, HEADS], F32)
    w_as_bf_d = inp("w_as_bf", [F_IN, HEADS], BF)
    W_heads_d = inp("W_heads", [F_IN, F_GAT], BF)
    b_gat_row_d = inp("b_gat_row", [1, F_GAT], BF)
    ones_row_d = inp("ones_row", [1, 128], BF)
    ident_d = inp("ident", [128, 128], BF)
    W_gcn_d = inp("W_gcn_p", [128, 6 * GCN_OUT], BF)
    W_g1_d = inp("W_g1_p", [128, 11 * 1000], BF)
    W_g2_d = inp("W_g2_p", [128, 8 * 64], BF)
    b_gcn_cols_d = inp("b_gcn_cols", [128, 11], F32)
    b_g1_cols_d = inp("b_g1_cols", [128, 8], F32)
    W_fc1_d = inp("W_fc1_p", [64, 32], BF)
    W_fc2_d = inp("W_fc2_p", [32, 16], BF)
    W_out_d = inp("W_out_p", [16, 1], BF)
    b_tail_d = inp("b_tail", [128, 4], F32)

    y_d = nc.dram_tensor("y", [1, SLOTS_PER_CORE], F32, kind="ExternalOutput")

    h_full = nc.dram_tensor("h_full", [NSLOT, HW_H], BF)
    # quarter boundaries: tile index ranges and h_full row offsets
    q_of_tile = []
    for qi, qt in enumerate(QT):
        q_of_tile += [qi] * qt
    q_last_tile = [sum(QT[:i + 1]) - 1 for i in range(len(QT))]
    q_base = [NCORE * 128 * sum(QT[:i]) for i in range(len(QT))]
    hdump_d = (nc.dram_tensor("hdump", [NSLOT, HW_H], BF,
                              kind="ExternalOutput") if dump_h else None)

    core_ids = list(range(NCORE))
    AF = mybir.ActivationFunctionType
    OP = mybir.AluOpType

    with tile.TileContext(nc) as tc:
        with tc.tile_pool(name="persist", bufs=1) as pp, \
             tc.tile_pool(name="dram", bufs=1, space="DRAM") as dram:

            h_myQ = [dram.tile([qt * 128, HW_H], BF, tag=f"h_myQ{qi}",
                               name=f"h_myQ{qi}")
                     for qi, qt in enumerate(QT)]

            sidx_sb = pp.tile([128, TILES_PER_CORE * nc_t * 8], I16)
            nc.gpsimd.load_library(mlp_lib)
            ident_sb = pp.tile([128, 128], BF)
            alpha02 = pp.tile([128, 1], F32)
            ad_my = pp.tile([128, TILES_PER_CORE * HEADS], BF)
            nc.sync.dma_start(sidx_sb[:], idx16_d[:])
            nc.sync.dma_start(ident_sb[:], ident_d[:])
            nc.vector.memset(alpha02[:], 0.2)

            for _rep in range(repeat):
                # ---------------- phase A: per-dst-node a_d coefficients ----
                if "A" in phases:
                    with tc.tile_pool(name="phaseA", bufs=1) as pa, \
                         tc.tile_pool(name="psumA", bufs=4,
                                      space=bass.MemorySpace.PSUM) as psa:
                        xTmy_sb = pa.tile([F_IN, SLOTS_PER_CORE], F32)
                        nc.sync.dma_start(xTmy_sb[:], xTmy_d[:])
                        wad_sb = pa.tile([F_IN, HEADS], F32)
                        nc.sync.dma_start(wad_sb[:], w_ad_d[:])
                        for t in range(TILES_PER_CORE):
                            ps = psa.tile([128, HEADS], F32, tag="psA")
                            nc.tensor.matmul(ps[:],
                                             xTmy_sb[:, 128 * t:128 * (t + 1)],
                                             wad_sb[:], start=True, stop=True)
                            nc.vector.tensor_copy(
                                ad_my[:, 10 * t:10 * (t + 1)], ps[:])

                with tc.tile_pool(name="earlyC", bufs=1) as pec:
                    # phase-C weights stream in on the ACT queue during B;
                    # DMA issues are deferred to mid-phase-B (t==3) so they
                    # don't starve B's own streams at kernel start.
                    wmasks_sb = pec.tile([128, nchunks * 128], BF)
                    W_gcn_sb = pec.tile([128, 6 * GCN_OUT], BF)
                    W_g1_sb = pec.tile([128, 11 * 1000], BF)
                    W_g2_sb = pec.tile([128, 8 * 64], BF)
                    b_gcn_sb = pec.tile([128, 11], F32)
                    b_g1_sb = pec.tile([128, 8], F32)
                    W_fc1_sb = pec.tile([64, 32], BF)
                    W_fc2_sb = pec.tile([32, 16], BF)
                    W_out_sb = pec.tile([16, 1], BF)
                    b_tail_sb = pec.tile([128, 4], F32)

                    def _issue_earlyC():
                        nc.scalar.dma_start(wmasks_sb[:], wmasks_d[:])
                        nc.scalar.dma_start(W_gcn_sb[:], W_gcn_d[:])
                        nc.scalar.dma_start(W_g1_sb[:], W_g1_d[:])
                        nc.scalar.dma_start(W_g2_sb[:], W_g2_d[:])
                        nc.scalar.dma_start(b_gcn_sb[:], b_gcn_cols_d[:])
                        nc.scalar.dma_start(b_g1_sb[:], b_g1_cols_d[:])
                        nc.scalar.dma_start(W_fc1_sb[:], W_fc1_d[:])
                        nc.scalar.dma_start(W_fc2_sb[:], W_fc2_d[:])
                        nc.scalar.dma_start(W_out_sb[:], W_out_d[:])
                        nc.scalar.dma_start(b_tail_sb[:], b_tail_d[:])

                    if "B" not in phases:
                        _issue_earlyC()
                    # ---------------- phase B: GAT ----------------
                    if "B" in phases:
                        with tc.tile_pool(name="phaseBw", bufs=1) as pbw, \
                             tc.tile_pool(name="bstream", bufs=2) as pbs, \
                             tc.tile_pool(name="gat_work", bufs=6) as gw, \
                             tc.tile_pool(name="gat_tail", bufs=2) as gt, \
                             tc.tile_pool(name="psumG", bufs=1,
                                          space=bass.MemorySpace.PSUM) as psg, \
                             tc.tile_pool(name="psumE", bufs=1,
                                          space=bass.MemorySpace.PSUM) as pse, \
                             tc.tile_pool(name="psumH", bufs=1,
                                          space=bass.MemorySpace.PSUM) as psh, \
                             tc.tile_pool(name="psumT", bufs=2,
                                          space=bass.MemorySpace.PSUM) as pst:

                            was_sb = pbw.tile([F_IN, HEADS], BF)
                            nc.sync.dma_start(was_sb[:], w_as_bf_d[:])
                            W_heads_sb = pbw.tile([F_IN, F_GAT], BF)
                            nc.sync.dma_start(W_heads_sb[:], W_heads_d[:])
                            b_gat_sb = pbw.tile([1, F_GAT], BF)
                            nc.sync.dma_start(b_gat_sb[:], b_gat_row_d[:])
                            ones_sb = pbw.tile([1, 128], BF)
                            nc.sync.dma_start(ones_sb[:], ones_row_d[:])

                            ew = nc_t * 128
                            for t in range(TILES_PER_CORE):
                                # stream this tile's edge operands
                                xg_t = pbs.tile([128, nc_t * F_IN], BF, tag="xg")
                                nc.sync.dma_start(
                                    xg_t[:], xg_d[:, t * nc_t * F_IN:
                                                  (t + 1) * nc_t * F_IN])
                                xgT_t = pbs.tile([F_IN, ew], BF, tag="xgT")
                                nc.sync.dma_start(
                                    xgT_t[:], xgT_d[:, t * ew:(t + 1) * ew])
                                masks_t = pbs.tile([128, ew], BF, tag="masks")
                                nc.sync.dma_start(
                                    masks_t[:], masks_d[:, t * ew:(t + 1) * ew])
                                masksT_t = pbs.tile([128, ew], BF, tag="masksT")
                                nc.sync.dma_start(
                                    masksT_t[:], masksT_d[:, t * ew:(t + 1) * ew])

                                if t == 3:
                                    _issue_earlyC()
                                ad_tile = ad_my[:, 10 * t:10 * (t + 1)]
                                # psum_g: agg at [0:660), s at [660:670)
                                psum_g = psg.tile([128, 670], F32, tag="psg")
                                for k in range(nc_t):
                                    pse_t = pse.tile([128, HEADS], F32,
                                                     tag=f"pse{k % 2}",
                                                     name=f"pse{k % 2}")
                                    ps_e = pse_t[:]
                                    nc.tensor.matmul(
                                        ps_e, xgT_t[:, 128 * k:128 * (k + 1)],
                                        was_sb[:], start=True, stop=False)
                                    nc.tensor.matmul(
                                        ps_e, masksT_t[:, 128 * k:128 * (k + 1)],
                                        ad_tile, start=False, stop=True)
                                    ev = gw.tile([128, HEADS], F32, tag="ev")
                                    nc.scalar.activation(
                                        ev[:], ps_e, AF.Prelu,
                                        alpha=alpha02[:, 0:1])
                                    rhs = gw.tile([128, 670], BF, tag="rhs")
                                    nc.scalar.activation(
                                        rhs[:, 660:670], ev[:], AF.Exp)
                                    xg_b = _bc(xg_t[:, F_IN * k:F_IN * k + 1],
                                               [[0, HEADS], [1, F_HEAD]])
                                    ex_b = _bc(rhs[:, 660:661],
                                               [[1, HEADS], [0, F_HEAD]])
                                    veng = nc.gpsimd if k % 4 == 3 else nc.vector
                                    veng.tensor_tensor(
                                        rhs[:, 0:F_GAT].rearrange(
                                            "p (h f) -> p h f", h=HEADS),
                                        xg_b, ex_b, OP.mult)
                                    mask = masks_t[:, 128 * k:128 * (k + 1)]
                                    st, sp = (k == 0), (k == nc_t - 1)
                                    nc.tensor.matmul(psum_g[:, 0:512], mask,
                                                     rhs[:, 0:512],
                                                     start=st, stop=sp)
                                    nc.tensor.matmul(psum_g[:, 512:670], mask,
                                                     rhs[:, 512:670],
                                                     start=st, stop=sp)

                                # ---- tile tail: normalize, per-head W, bias+leaky
                                s_sb = gw.tile([128, HEADS], F32, tag="s")
                                nc.vector.tensor_scalar(s_sb[:],
                                                        psum_g[:, 660:670],
                                                        1e-6, None, OP.max)
                                rs = gw.tile([128, HEADS], F32, tag="rs")
                                nc.vector.reciprocal(rs[:], s_sb[:])
                                # A_norm: head h at cols [128h, 128h+66),
                                # zero-padded to 128 so transposed blocks are
                                # partition-0 aligned for the per-head matmul.
                                A_norm = gt.tile([128, 1280], BF, tag="anorm")
                                if t < 2:
                                    for h in range(HEADS):
                                        nc.vector.memset(
                                            A_norm[:, 128 * h + 66:
                                                   128 * (h + 1)], 0.0)
                                rs_b = _bc(rs[:, 0:1], [[1, HEADS], [0, F_HEAD]])
                                nc.vector.tensor_tensor(
                                    _bc(A_norm[:, 0:1],
                                        [[128, HEADS], [1, F_HEAD]]),
                                    _bc(psum_g[:, 0:1], [[66, HEADS], [1, F_HEAD]]),
                                    rs_b, OP.mult)
                                psum_h1 = psh.tile([128, 330], F32, tag="psh1")
                                psum_h2 = psh.tile([128, 330], F32, tag="psh2")
                                for h in range(HEADS):
                                    ph = psum_h1 if h < 5 else psum_h2
                                    o = 66 * h - (0 if h < 5 else 330)
                                    tp = pst.tile([96, 128], BF, tag="tp")
                                    nc.tensor.transpose(
                                        tp[:], A_norm[:, 128 * h:128 * h + 96],
                                        ident_sb[:])
                                    ahT = gw.tile([96, 128], BF, tag="ahT")
                                    nc.vector.tensor_copy(ahT[:], tp[:])
                                    nc.tensor.matmul(
                                        ph[:, o:o + 66], ahT[0:66, :],
                                        W_heads_sb[:, 66 * h:66 * (h + 1)],
                                        start=True, stop=False)
                                    nc.tensor.matmul(
                                        ph[:, o:o + 66], ones_sb[0:1, 0:128],
                                        b_gat_sb[0:1, 66 * h:66 * (h + 1)],
                                        start=False, stop=True)
                                h_tile = gt.tile([128, HW_H], BF, tag="htile")
                                if t < 2:
                                    nc.vector.memset(h_tile[:, 660:HW_H], 0.0)
                                for half, phs in ((0, psum_h1), (1, psum_h2)):
                                    nc.scalar.activation(
                                        h_tile[:, 330 * half:330 * (half + 1)],
                                        phs[:, 0:330], AF.Lrelu)
                                qi = q_of_tile[t]
                                tt = t - sum(QT[:qi])
                                nc.gpsimd.dma_start(
                                    h_myQ[qi][128 * tt:128 * (tt + 1), :],
                                    h_tile[:])
                                if (SPLIT_AG and "G" in phases
                                        and t == q_last_tile[qi]
                                        and qi < len(QT) - 1):
                                    nc.gpsimd.collective_compute(
                                        "AllGather", OP.bypass,
                                        replica_groups=[core_ids],
                                        ins=[h_myQ[qi].opt()],
                                        outs=[h_full[q_base[qi]:
                                                     q_base[qi] + NCORE * 128
                                                     * QT[qi], :]])

                    # ---------------- AllGather (remaining quarters) --------
                    if "G" in phases:
                        qs = ([len(QT) - 1] if SPLIT_AG
                              else list(range(len(QT))))
                        for qi in qs:
                            nc.gpsimd.collective_compute(
                                "AllGather", OP.bypass,
                                replica_groups=[core_ids],
                                ins=[h_myQ[qi].opt()],
                                outs=[h_full[q_base[qi]:
                                             q_base[qi] + NCORE * 128
                                             * QT[qi], :]])

                    if dump_h:
                        nc.gpsimd.dma_start(hdump_d[:], h_full[:])

                    # ---------------- phase C: GCN + feature-major MLP ----------
                    if "C" in phases:
                        with tc.tile_pool(name="hg_pool", bufs=3) as hgp, \
                             tc.tile_pool(name="gcn_work", bufs=4) as gcw, \
                             tc.tile_pool(name="grp", bufs=2) as grp, \
                             tc.tile_pool(name="psumC", bufs=2,
                                          space=bass.MemorySpace.PSUM) as psc, \
                             tc.tile_pool(name="psumT2", bufs=2,
                                          space=bass.MemorySpace.PSUM) as pst2, \
                             tc.tile_pool(name="psumM", bufs=2,
                                          space=bass.MemorySpace.PSUM) as psm:

                            kws = [128] * 5 + [32]
                            kws1 = [128] * 10 + [40]
                            kws2 = [128] * 7 + [104]
                            groups = [(0, 4), (4, 4), (8, 2)]

                            def dense_stack(g0, gn, aggT):
                                nw = gn * 128
                                gT = grp.tile([128, 11 * 512], BF, tag="gT")
                                for mc in range(11):
                                    mw = 128 if mc < 10 else 40
                                    ps = psm.tile([128, 512], F32, tag="psm")
                                    for kt in range(6):
                                        nc.tensor.matmul(
                                            ps[0:mw, 0:nw],
                                            W_gcn_sb[0:kws[kt],
                                                     GCN_OUT * kt + 128 * mc:
                                                     GCN_OUT * kt + 128 * mc + mw],
                                            aggT[0:kws[kt],
                                                 512 * kt:512 * kt + nw],
                                            start=(kt == 0), stop=(kt == 5))
                                    nc.scalar.activation(
                                        gT[0:mw, 512 * mc:512 * mc + nw],
                                        ps[0:mw, 0:nw], AF.Lrelu,
                                        bias=b_gcn_sb[0:mw, mc:mc + 1])

                                z1T = grp.tile([128, 8 * 512], BF, tag="z1T")
                                for mc in range(8):
                                    mw = 128 if mc < 7 else 104
                                    ps = psm.tile([128, 512], F32, tag="psm")
                                    for kt in range(11):
                                        nc.tensor.matmul(
                                            ps[0:mw, 0:nw],
                                            W_g1_sb[0:kws1[kt],
                                                    1000 * kt + 128 * mc:
                                                    1000 * kt + 128 * mc + mw],
                                            gT[0:kws1[kt],
                                               512 * kt:512 * kt + nw],
                                            start=(kt == 0), stop=(kt == 10))
                                    nc.scalar.activation(
                                        z1T[0:mw, 512 * mc:512 * mc + nw],
                                        ps[0:mw, 0:nw], AF.Lrelu,
                                        bias=b_g1_sb[0:mw, mc:mc + 1])

                                ps2 = psm.tile([128, 512], F32, tag="psm")
                                for kt in range(8):
                                    nc.tensor.matmul(
                                        ps2[0:64, 0:nw],
                                        W_g2_sb[0:kws2[kt], 64 * kt:64 * kt + 64],
                                        z1T[0:kws2[kt], 512 * kt:512 * kt + nw],
                                        start=(kt == 0), stop=(kt == 7))
                                z2T = gcw.tile([64, 512], BF, tag="z2T")
                                nc.scalar.activation(z2T[:, 0:nw], ps2[0:64, 0:nw],
                                                     AF.Lrelu,
                                                     bias=b_tail_sb[0:64, 0:1])

                                ps3 = psm.tile([128, 512], F32, tag="psm")
                                nc.tensor.matmul(ps3[0:32, 0:nw], W_fc1_sb[:],
                                                 z2T[0:64, 0:nw],
                                                 start=True, stop=True)
                                z3T = gcw.tile([32, 512], BF, tag="z3T")
                                nc.scalar.activation(z3T[:, 0:nw], ps3[0:32, 0:nw],
                                                     AF.Lrelu,
                                                     bias=b_tail_sb[0:32, 1:2])

                                ps4 = psm.tile([128, 512], F32, tag="psm")
                                nc.tensor.matmul(ps4[0:16, 0:nw], W_fc2_sb[:],
                                                 z3T[0:32, 0:nw],
                                                 start=True, stop=True)
                                z4T = gcw.tile([16, 512], BF, tag="z4T")
                                nc.scalar.activation(z4T[:, 0:nw], ps4[0:16, 0:nw],
                                                     AF.Lrelu,
                                                     bias=b_tail_sb[0:16, 2:3])

                                ps5 = psm.tile([128, 512], F32, tag="psm")
                                nc.tensor.matmul(ps5[0:1, 0:nw], W_out_sb[:],
                                                 z4T[0:16, 0:nw],
                                                 start=True, stop=True)
                                outT = gcw.tile([1, 512], F32, tag="outT")
                                nc.scalar.activation(outT[0:1, 0:nw],
                                                     ps5[0:1, 0:nw], AF.Identity,
                                                     bias=b_tail_sb[0:1, 3:4])
                                nc.sync.dma_start(
                                    y_d[0:1, 128 * g0:128 * g0 + nw],
                                    outT[0:1, 0:nw])

                            for g0, gn in groups:
                                aggT = grp.tile([128, 6 * 512], BF, tag="aggT")
                                for j in range(gn):
                                    t = g0 + j
                                    psum_a = psc.tile([128, F_GAT], F32, tag="psa")
                                    hg = hgp.tile([128, nc_t * HW_H], BF,
                                                  tag="hg")
                                    h1 = (nc_t + 1) // 2
                                    o = t * nc_t * 8
                                    for (ka, kb) in ((0, h1), (h1, nc_t)):
                                        nidx = (kb - ka) * 128
                                        nc.gpsimd.dma_gather(
                                            hg[:, ka * HW_H:kb * HW_H]
                                            .rearrange("p (k w) -> p k w",
                                                       w=HW_H),
                                            h_full[:],
                                            sidx_sb[:, o + ka * 8:o + kb * 8],
                                            num_idxs=nidx,
                                            num_idxs_reg=nidx,
                                            elem_size=HW_H,
                                            queue_num=0)
                                    for k in range(nc_t):
                                        c = t * nc_t + k
                                        wmask = wmasks_sb[:, 128 * c:128 * (c + 1)]
                                        st, sp = (k == 0), (k == nc_t - 1)
                                        nc.tensor.matmul(
                                            psum_a[:, 0:512], wmask,
                                            hg[:, HW_H * k:HW_H * k + 512],
                                            start=st, stop=sp)
                                        nc.tensor.matmul(
                                            psum_a[:, 512:660], wmask,
                                            hg[:, HW_H * k + 512:HW_H * k + 660],
                                            start=st, stop=sp)
                                    agg = gcw.tile([128, 768], BF, tag="agg")
                                    nc.vector.tensor_copy(agg[:, 0:660],
                                                          psum_a[:, 0:660])
                                    nc.vector.memset(agg[:, 660:768], 0.0)
                                    for b in range(6):
                                        w = kws[b]
                                        tp2 = pst2.tile([128, 128], BF,
                                                        tag="tp2")
                                        nc.tensor.transpose(
                                            tp2[0:w, :],
                                            agg[:, 128 * b:128 * b + w],
                                            ident_sb[:])
                                        nc.vector.tensor_copy(
                                            aggT[0:w, 512 * b + 128 * j:
                                                 512 * b + 128 * (j + 1)],
                                            tp2[0:w, :])
                                dense_stack(g0, gn, aggT)

    nc.compile()
    return nc


# ---------------------------------------------------------------- entry point

def kernel(x, edge_index, W_gat, att_src, att_dst, b_gat, W_gcn, b_gcn,
           W_g1, b_g1, W_g2, b_g2, W_fc1, b_fc1, W_fc2, b_fc2, W_out, b_out,
           _want_trace=False):
    x = np.asarray(x, np.float32)
    edge_index = np.asarray(edge_index)
    prep = _prep(x, edge_index)
    wts = _prep_weights(W_gat, att_src, att_dst, b_gat, W_gcn, b_gcn,
                        W_g1, b_g1, W_g2, b_g2, W_fc1, b_fc1, W_fc2, b_fc2,
                        W_out, b_out)

    nc_t = prep["nc_t"]
    if nc_t not in _CACHE:
        _CACHE[nc_t] = _build(nc_t)
    nc = _CACHE[nc_t]

    in_maps = _in_maps(prep, wts)
    res = run_bass_kernel_spmd(nc, in_maps, list(range(NCORE)),
                               trace=_want_trace)
    y_all = np.concatenate([np.asarray(res.results[c]["y"]).reshape(-1)
                            for c in range(NCORE)])
    out = y_all[prep["slot"]].astype(np.float32).reshape(N, 1)
    if _want_trace:
        return out, res
    return out


def _in_maps(prep, wts):
    shared = dict(
        w_ad=wts["w_ad"], w_as_bf=wts["w_as_bf"], W_heads=wts["W_heads"],
        b_gat_row=wts["b_gat_row"], ones_row=wts["ones_row"],
        ident=wts["ident"],
        W_gcn_p=wts["W_gcn_p"], W_g1_p=wts["W_g1_p"], W_g2_p=wts["W_g2_p"],
        b_gcn_cols=wts["b_gcn_cols"], b_g1_cols=wts["b_g1_cols"],
        W_fc1_p=wts["W_fc1_p"], W_fc2_p=wts["W_fc2_p"],
        W_out_p=wts["W_out_p"], b_tail=wts["b_tail"])
    in_maps = []
    for c in range(NCORE):
        m = dict(shared)
        for k2 in ["xg", "xgT", "masks", "masksT", "wmasks", "idx16", "xT_my"]:
            m[k2] = prep[k2][c]
        in_maps.append(m)
    return in_maps


if __name__ == "__main__":
    sys.path.insert(0, os.path.dirname(os.path.abspath(__file__)))
    import reference
    inputs = reference.setup_inputs()
    inputs = {k: np.asarray(v) for k, v in inputs.items()}
    expected = np.asarray(reference.reference(**inputs))
    got = kernel(**inputs)
    err = np.linalg.norm(got - expected) / np.linalg.norm(expected)
    print("Relative error:", err)



# revision 13
# speedup vs baseline: 1.0503x; 1.0503x over previous
"""Trainium2 Bass kernel for GAT+GCN+MLP message passing (8 NeuronCores, SPMD).

Strategy (dst-node sharding), v2:
  - Host: add self-loops, pack the 10000 nodes into 8 cores x 10 tiles x 128
    slots balancing in-edge counts; build per-edge one-hot dst masks (plus
    transposed and GCN-norm-weighted variants) and edge-expanded x operands.
  - GAT (phase B): per edge-chunk, e-values via matmuls accumulated in spare
    PSUM columns; leaky(0.2) via Prelu on the scalar engine; exp written
    straight into the message-rhs tile; messages (x_src * ex, 10 heads via
    broadcast APs) aggregated per dst tile with one-hot mask matmuls into
    PSUM together with the softmax denominators; normalization via one
    broadcast DVE multiply; per-head W transform via transposes; bias +
    leaky(0.01) fused into a fixed-slope Lrelu activation readout.
  - AllGather of the hidden h (768-wide bf16) split into QT quarters; the
    first quarters are issued mid-phase-B so the collective overlaps GAT
    compute and only the final 1-tile quarter is exposed.
  - GCN+MLP (phase C): h rows fetched with batched gpsimd dma_gather calls
    (int16 wrapped indices, half-tile granularity to fit the SWDGE ring);
    aggregation with norm-weighted mask matmuls; dense stack feature-major
    with weights stationary; every dense readout is a single Lrelu
    activation with the bias applied on the partition axis. Phase-C weights
    stream in mid-phase-B on the ACT DMA queue.
"""

import os
import sys
import heapq
import dataclasses

for _p in ("/opt/trn_rl_repo", "/root/.axon_site/_ro/trn_rl_repo"):
    if os.path.isdir(_p) and _p not in sys.path:
        sys.path.insert(0, _p)

import numpy as np
import ml_dtypes

import concourse.bass as bass
import concourse.tile as tile
from concourse import bacc, mybir
from concourse.bass_utils import run_bass_kernel_spmd
from concourse.library_config import mlp as mlp_lib

BF16 = ml_dtypes.bfloat16

N = 10000
F_IN = 66
HEADS = 10
F_HEAD = 66
F_GAT = HEADS * F_HEAD          # 660
GCN_OUT = 1320
NCORE = 8
TILES_PER_CORE = 10
NTILE = NCORE * TILES_PER_CORE  # 80
NSLOT = NTILE * 128             # 10240
SLOTS_PER_CORE = TILES_PER_CORE * 128  # 1280

F32 = mybir.dt.float32
BF = mybir.dt.bfloat16
I32 = mybir.dt.int32
I16 = mybir.dt.int16
HW_H = 768   # h row width for dma_gather (256B-multiple); cols 660:768 zero
QT = [3, 3, 3, 1]   # tiles per AllGather quarter (split collective)

_CACHE = {}
SPLIT_AG = os.environ.get("KERNEL_SPLIT_AG", "1") == "1"
SHARED_H = os.environ.get("KERNEL_SHARED_H", "1") == "1"
NSWQ = int(os.environ.get("KERNEL_NSWQ", "2"))
BATCH_E = os.environ.get("KERNEL_BATCH_E", "1") == "1"
TAGG = os.environ.get("KERNEL_TAGG", "1") == "1"
AHT_SCALAR = os.environ.get("KERNEL_AHT_SCALAR", "1") == "1"


# ---------------------------------------------------------------- host prep

def _prep(x, edge_index):
    src = np.concatenate([edge_index[0], np.arange(N, dtype=np.int64)])
    dst = np.concatenate([edge_index[1], np.arange(N, dtype=np.int64)])
    deg = np.bincount(dst, minlength=N).astype(np.int64)

    # pack nodes into 80 tiles of <=128 slots, balancing in-edge counts
    order = np.argsort(-deg, kind="stable")
    tile_cnt = np.zeros(NTILE, np.int64)
    slot = np.empty(N, np.int64)
    hp = [(0, t) for t in range(NTILE)]
    heapq.heapify(hp)
    for n_ in order:
        while True:
            e, t = heapq.heappop(hp)
            if tile_cnt[t] < 128:
                break
        slot[n_] = t * 128 + tile_cnt[t]
        tile_cnt[t] += 1
        heapq.heappush(hp, (e + int(deg[n_]), t))
    sslot = slot[src]
    dslot = slot[dst]
    dtile = dslot >> 7
    dlocal = dslot & 127

    tile_edges = np.bincount(dtile, minlength=NTILE)
    nc_t = int(np.max((tile_edges + 127) // 128))
    e_tile = nc_t * 128
    nchunks = TILES_PER_CORE * nc_t

    esrc = np.zeros((NCORE, TILES_PER_CORE, e_tile), np.int64)
    edstl = np.full((NCORE, TILES_PER_CORE, e_tile), -1, np.int64)
    edst = np.zeros((NCORE, TILES_PER_CORE, e_tile), np.int64)
    ord_t = np.argsort(dtile, kind="stable")
    bounds = np.searchsorted(dtile[ord_t], np.arange(NTILE + 1))
    for t in range(NTILE):
        idx = ord_t[bounds[t]:bounds[t + 1]]
        k = len(idx)
        c, tt = divmod(t, TILES_PER_CORE)
        esrc[c, tt, :k] = sslot[idx]
        edstl[c, tt, :k] = dlocal[idx]
        edst[c, tt, :k] = dslot[idx]

    # one-hot masks [core][128 dst-part, nchunks*128 edges] (+ transposed)
    onehot = (edstl[..., None] == np.arange(128))      # [C,T,e_tile,128] bool
    oh = onehot.reshape(NCORE, TILES_PER_CORE, nc_t, 128, 128)
    masks = np.ascontiguousarray(
        oh.transpose(0, 3, 1, 2, 4)).reshape(
        NCORE, 128, nchunks * 128).astype(BF16)
    masksT = np.ascontiguousarray(
        oh.transpose(0, 4, 1, 2, 3)).reshape(
        NCORE, 128, nchunks * 128).astype(BF16)

    # norm-weighted masks for GCN: w = dinv[src]*dinv[dst] folded in
    dinv_slot = np.ones(NSLOT, np.float32)
    dinv_slot[slot] = 1.0 / np.sqrt(np.maximum(deg, 1).astype(np.float32))
    wvals = (dinv_slot[esrc] * dinv_slot[edst]).astype(np.float32)
    wm = oh.astype(np.float32) * wvals.reshape(
        NCORE, TILES_PER_CORE, nc_t, 128)[..., None]
    wmasks = np.ascontiguousarray(
        wm.transpose(0, 3, 1, 2, 4)).reshape(
        NCORE, 128, nchunks * 128).astype(BF16)

    # edge-expanded x operands
    x_pad = np.zeros((NSLOT, F_IN), np.float32)
    x_pad[slot] = x
    xg = np.empty((NCORE, 128, nchunks * F_IN), BF16)
    xgT = np.empty((NCORE, F_IN, nchunks * 128), BF16)
    for c in range(NCORE):
        arr = x_pad[esrc[c].reshape(-1)]               # [nidx, 66] f32
        a3 = arr.reshape(nchunks, 128, F_IN)
        xg[c] = np.ascontiguousarray(
            a3.transpose(1, 0, 2)).reshape(128, nchunks * F_IN).astype(BF16)
        xgT[c] = np.ascontiguousarray(arr.T).astype(BF16)

    # gather-row remap for the split AllGather: h_full row order is
    # [all cores' Q1 tiles | Q2 | Q3 | Q4] with quarters of QT tiles each
    sc = np.arange(NSLOT) // SLOTS_PER_CORE
    loc = np.arange(NSLOT) % SLOTS_PER_CORE
    row_of_slot = np.zeros(NSLOT, np.int64)
    lo = 0
    base = 0
    for qt in QT:
        rows = qt * 128
        m = (loc >= lo) & (loc < lo + rows)
        row_of_slot[m] = base + sc[m] * rows + (loc[m] - lo)
        lo += rows
        base += NCORE * rows
    # dma_gather index layout: per (tile, half-tile) block, flat order
    # i = chunk*128 + partition, wrapped as idx16[i % 16, i // 16] and
    # replicated down all 128 partitions. Half-tile granularity keeps each
    # gather under the 1024-descriptor SWDGE ring.
    nit = nc_t * 128                       # idxs per tile
    h1 = (nc_t + 1) // 2                   # chunks in first half
    idx16 = np.empty((NCORE, 128, TILES_PER_CORE * (nit // 16)), np.int16)
    for c in range(NCORE):
        rows = row_of_slot[esrc[c]]        # [T, nc_t*128] (chunk-major flat)
        for t in range(TILES_PER_CORE):
            o = t * (nit // 16)
            for (a, b) in ((0, h1 * 128), (h1 * 128, nit)):
                flat = rows[t][a:b]
                n = b - a
                w16 = np.zeros((16, n // 16), np.int16)
                w16[np.arange(n) % 16, np.arange(n) // 16] = flat
                idx16[c, :, o + a // 16:o + b // 16] = np.tile(w16, (8, 1))

    xT = np.zeros((F_IN, NSLOT), np.float32)
    xT[:, slot] = x.T
    xT_my = np.stack([np.ascontiguousarray(
        xT[:, c * SLOTS_PER_CORE:(c + 1) * SLOTS_PER_CORE])
        for c in range(NCORE)])

    return dict(slot=slot, nc_t=nc_t, masks=masks, masksT=masksT,
                wmasks=wmasks, xg=xg, xgT=xgT, idx16=idx16, xT_my=xT_my)


def _prep_weights(W_gat, att_src, att_dst, b_gat, W_gcn, b_gcn,
                  W_g1, b_g1, W_g2, b_g2, W_fc1, b_fc1, W_fc2, b_fc2,
                  W_out, b_out):
    Wg = np.asarray(W_gat, np.float32).reshape(F_IN, HEADS, F_HEAD)
    w_as = np.einsum("fhg,hg->fh", Wg, np.asarray(att_src, np.float32))
    w_ad = np.einsum("fhg,hg->fh", Wg, np.asarray(att_dst, np.float32))
    w_ad = np.ascontiguousarray(w_ad.astype(np.float32))          # [66,10]
    w_as_bf = np.ascontiguousarray(w_as.astype(BF16))             # [66,10]

    def chunk_pack(W, kchunks, ncols):
        W = np.asarray(W, np.float32)
        K, M = W.shape
        out = np.zeros((128, kchunks * ncols), BF16)
        for kt in range(kchunks):
            r0 = kt * 128
            r1 = min(K, r0 + 128)
            if r0 >= K:
                break
            out[:r1 - r0, kt * ncols:kt * ncols + M] = W[r0:r1].astype(BF16)
        return out

    W_gcn_p = chunk_pack(W_gcn, 6, GCN_OUT)
    W_g1_p = chunk_pack(W_g1, 11, 1000)
    W_g2_p = chunk_pack(W_g2, 8, 64)

    def col_pack(b, nch):
        b = np.asarray(b, np.float32).reshape(-1)
        out = np.zeros((nch, 128), np.float32)
        out.reshape(-1)[:b.shape[0]] = b
        return np.ascontiguousarray(out.T)

    b_gcn_cols = col_pack(b_gcn, 11)                  # [128, 11]
    b_g1_cols = col_pack(b_g1, 8)                     # [128, 8]
    W_fc1_p = np.asarray(W_fc1, BF16)
    W_fc2_p = np.asarray(W_fc2, BF16)
    W_out_p = np.asarray(W_out, BF16)
    b_tail = np.zeros((128, 4), np.float32)
    b_tail[:64, 0] = np.asarray(b_g2, np.float32)
    b_tail[:32, 1] = np.asarray(b_fc1, np.float32)
    b_tail[:16, 2] = np.asarray(b_fc2, np.float32)
    b_tail[0, 3] = float(np.asarray(b_out).reshape(-1)[0])

    ident = np.eye(128, dtype=BF16)
    ones_row = np.ones((1, 128), BF16)
    b_gat_row = np.zeros((1, F_GAT), BF16)
    b_gat_row[0, :] = np.asarray(b_gat, BF16)
    W_heads = np.asarray(W_gat, BF16)

    return dict(w_ad=w_ad, w_as_bf=w_as_bf, W_gcn_p=W_gcn_p, W_g1_p=W_g1_p,
                W_g2_p=W_g2_p, b_gcn_cols=b_gcn_cols, b_g1_cols=b_g1_cols,
                W_fc1_p=W_fc1_p, W_fc2_p=W_fc2_p, W_out_p=W_out_p,
                b_tail=b_tail, ident=ident, b_gat_row=b_gat_row,
                ones_row=ones_row, W_heads=W_heads)


# ---------------------------------------------------------------- device kernel

def _bc(ap, pattern):
    """Replace the free dims of a (sliced) AP with explicit [step,count] dims."""
    return dataclasses.replace(
        ap, ap=[list(ap.ap[0])] + [list(p) for p in pattern])


def _build(nc_t, repeat=1, phases="ABGC", dump_h=False):
    nchunks = TILES_PER_CORE * nc_t

    nc = bacc.Bacc("TRN2", target_bir_lowering=False, debug=False,
                   num_devices=NCORE, num_swdge_queues=NSWQ)

    def inp(name, shape, dt):
        return nc.dram_tensor(name, list(shape), dt, kind="ExternalInput")

    xg_d = inp("xg", [128, nchunks * F_IN], BF)
    xgT_d = inp("xgT", [F_IN, nchunks * 128], BF)
    masks_d = inp("masks", [128, nchunks * 128], BF)
    masksT_d = inp("masksT", [128, nchunks * 128], BF)
    wmasks_d = inp("wmasks", [128, nchunks * 128], BF)
    idx16_d = inp("idx16", [128, TILES_PER_CORE * nc_t * 8], I16)
    xTmy_d = inp("xT_my", [F_IN, SLOTS_PER_CORE], F32)
    w_ad_d = inp("w_ad", [F_IN, HEADS], F32)
    w_as_bf_d = inp("w_as_bf", [F_IN, HEADS], BF)
    W_heads_d = inp("W_heads", [F_IN, F_GAT], BF)
    b_gat_row_d = inp("b_gat_row", [1, F_GAT], BF)
    ones_row_d = inp("ones_row", [1, 128], BF)
    ident_d = inp("ident", [128, 128], BF)
    W_gcn_d = inp("W_gcn_p", [128, 6 * GCN_OUT], BF)
    W_g1_d = inp("W_g1_p", [128, 11 * 1000], BF)
    W_g2_d = inp("W_g2_p", [128, 8 * 64], BF)
    b_gcn_cols_d = inp("b_gcn_cols", [128, 11], F32)
    b_g1_cols_d = inp("b_g1_cols", [128, 8], F32)
    W_fc1_d = inp("W_fc1_p", [64, 32], BF)
    W_fc2_d = inp("W_fc2_p", [32, 16], BF)
    W_out_d = inp("W_out_p", [16, 1], BF)
    b_tail_d = inp("b_tail", [128, 4], F32)

    y_d = nc.dram_tensor("y", [1, SLOTS_PER_CORE], F32, kind="ExternalOutput")

    h_full = nc.dram_tensor("h_full", [NSLOT, HW_H], BF,
                            addr_space=("Shared" if SHARED_H else "Local"))
    # quarter boundaries: tile index ranges and h_full row offsets
    q_of_tile = []
    for qi, qt in enumerate(QT):
        q_of_tile += [qi] * qt
    q_last_tile = [sum(QT[:i + 1]) - 1 for i in range(len(QT))]
    q_base = [NCORE * 128 * sum(QT[:i]) for i in range(len(QT))]
    hdump_d = (nc.dram_tensor("hdump", [NSLOT, HW_H], BF,
                              kind="ExternalOutput") if dump_h else None)

    core_ids = list(range(NCORE))
    AF = mybir.ActivationFunctionType
    OP = mybir.AluOpType

    with tile.TileContext(nc) as tc:
        with tc.tile_pool(name="persist", bufs=1) as pp, \
             tc.tile_pool(name="dram", bufs=1, space="DRAM") as dram:

            h_myQ = [dram.tile([qt * 128, HW_H], BF, tag=f"h_myQ{qi}",
                               name=f"h_myQ{qi}")
                     for qi, qt in enumerate(QT)]

            sidx_sb = pp.tile([128, TILES_PER_CORE * nc_t * 8], I16)
            nc.gpsimd.load_library(mlp_lib)
            ident_sb = pp.tile([128, 128], BF)
            alpha02 = pp.tile([128, 1], F32)
            ad_my = pp.tile([128, TILES_PER_CORE * HEADS], BF)
            nc.sync.dma_start(sidx_sb[:], idx16_d[:])
            nc.sync.dma_start(ident_sb[:], ident_d[:])
            nc.vector.memset(alpha02[:], 0.2)

            for _rep in range(repeat):
                # ---------------- phase A: per-dst-node a_d coefficients ----
                if "A" in phases:
                    with tc.tile_pool(name="phaseA", bufs=1) as pa, \
                         tc.tile_pool(name="psumA", bufs=4,
                                      space=bass.MemorySpace.PSUM) as psa:
                        xTmy_sb = pa.tile([F_IN, SLOTS_PER_CORE], F32)
                        nc.sync.dma_start(xTmy_sb[:], xTmy_d[:])
                        wad_sb = pa.tile([F_IN, HEADS], F32)
                        nc.sync.dma_start(wad_sb[:], w_ad_d[:])
                        for t in range(TILES_PER_CORE):
                            ps = psa.tile([128, HEADS], F32, tag="psA")
                            nc.tensor.matmul(ps[:],
                                             xTmy_sb[:, 128 * t:128 * (t + 1)],
                                             wad_sb[:], start=True, stop=True)
                            nc.vector.tensor_copy(
                                ad_my[:, 10 * t:10 * (t + 1)], ps[:])

                with tc.tile_pool(name="earlyC", bufs=1) as pec:
                    # phase-C weights stream in on the ACT queue during B;
                    # DMA issues are deferred to mid-phase-B (t==3) so they
                    # don't starve B's own streams at kernel start.
                    wmasks_sb = pec.tile([128, nchunks * 128], BF)
                    W_gcn_sb = pec.tile([128, 6 * GCN_OUT], BF)
                    W_g1_sb = pec.tile([128, 11 * 1000], BF)
                    W_g2_sb = pec.tile([128, 8 * 64], BF)
                    b_gcn_sb = pec.tile([128, 11], F32)
                    b_g1_sb = pec.tile([128, 8], F32)
                    W_fc1_sb = pec.tile([64, 32], BF)
                    W_fc2_sb = pec.tile([32, 16], BF)
                    W_out_sb = pec.tile([16, 1], BF)
                    b_tail_sb = pec.tile([128, 4], F32)

                    def _issue_earlyC():
                        nc.scalar.dma_start(wmasks_sb[:], wmasks_d[:])
                        nc.scalar.dma_start(W_gcn_sb[:], W_gcn_d[:])
                        nc.scalar.dma_start(W_g1_sb[:], W_g1_d[:])
                        nc.scalar.dma_start(W_g2_sb[:], W_g2_d[:])
                        nc.scalar.dma_start(b_gcn_sb[:], b_gcn_cols_d[:])
                        nc.scalar.dma_start(b_g1_sb[:], b_g1_cols_d[:])
                        nc.scalar.dma_start(W_fc1_sb[:], W_fc1_d[:])
                        nc.scalar.dma_start(W_fc2_sb[:], W_fc2_d[:])
                        nc.scalar.dma_start(W_out_sb[:], W_out_d[:])
                        nc.scalar.dma_start(b_tail_sb[:], b_tail_d[:])

                    if "B" not in phases:
                        _issue_earlyC()
                    # ---------------- phase B: GAT ----------------
                    if "B" in phases:
                        with tc.tile_pool(name="phaseBw", bufs=1) as pbw, \
                             tc.tile_pool(name="bstream", bufs=2) as pbs, \
                             tc.tile_pool(name="gat_work", bufs=6) as gw, \
                             tc.tile_pool(name="gat_tail", bufs=2) as gt, \
                             tc.tile_pool(name="psumG", bufs=1,
                                          space=bass.MemorySpace.PSUM) as psg, \
                             tc.tile_pool(name="psumE", bufs=2,
                                          space=bass.MemorySpace.PSUM) as pse, \
                             tc.tile_pool(name="psumH", bufs=1,
                                          space=bass.MemorySpace.PSUM) as psh, \
                             tc.tile_pool(name="psumT", bufs=2,
                                          space=bass.MemorySpace.PSUM) as pst:

                            was_sb = pbw.tile([F_IN, HEADS], BF)
                            nc.sync.dma_start(was_sb[:], w_as_bf_d[:])
                            W_heads_sb = pbw.tile([F_IN, F_GAT], BF)
                            nc.sync.dma_start(W_heads_sb[:], W_heads_d[:])
                            b_gat_sb = pbw.tile([1, F_GAT], BF)
                            nc.sync.dma_start(b_gat_sb[:], b_gat_row_d[:])
                            ones_sb = pbw.tile([1, 128], BF)
                            nc.sync.dma_start(ones_sb[:], ones_row_d[:])

                            ew = nc_t * 128
                            for t in range(TILES_PER_CORE):
                                # stream this tile's edge operands
                                xg_t = pbs.tile([128, nc_t * F_IN], BF, tag="xg")
                                nc.sync.dma_start(
                                    xg_t[:], xg_d[:, t * nc_t * F_IN:
                                                  (t + 1) * nc_t * F_IN])
                                xgT_t = pbs.tile([F_IN, ew], BF, tag="xgT")
                                nc.sync.dma_start(
                                    xgT_t[:], xgT_d[:, t * ew:(t + 1) * ew])
                                masks_t = pbs.tile([128, ew], BF, tag="masks")
                                nc.sync.dma_start(
                                    masks_t[:], masks_d[:, t * ew:(t + 1) * ew])
                                masksT_t = pbs.tile([128, ew], BF, tag="masksT")
                                nc.sync.dma_start(
                                    masksT_t[:], masksT_d[:, t * ew:(t + 1) * ew])

                                if t == 3:
                                    _issue_earlyC()
                                ad_tile = ad_my[:, 10 * t:10 * (t + 1)]
                                psum_g = psg.tile(
                                    [128, 660 if BATCH_E else 670], F32,
                                    tag="psg")
                                if BATCH_E:
                                    # ---- e-values for the whole tile in one
                                    # PSUM batch: one Prelu + one Exp ----
                                    # cols [0:10*nc_t): e-values;
                                    # cols [10*nc_t : 10*nc_t+10): softmax
                                    # denominators (group starts only after
                                    # every e-group has stopped -> legal
                                    # sequential groups in one zero region)
                                    psum_e = pse.tile([128, HEADS * nc_t + HEADS],
                                                      F32, tag="pse")
                                    for k in range(nc_t):
                                        sl = psum_e[:, 10 * k:10 * (k + 1)]
                                        nc.tensor.matmul(
                                            sl, xgT_t[:, 128 * k:128 * (k + 1)],
                                            was_sb[:], start=True, stop=False)
                                        nc.tensor.matmul(
                                            sl, masksT_t[:, 128 * k:128 * (k + 1)],
                                            ad_tile, start=False, stop=True)
                                    ev = gw.tile([128, HEADS * nc_t], F32,
                                                 tag="ev")
                                    nc.scalar.activation(
                                        ev[:], psum_e[:, 0:HEADS * nc_t],
                                        AF.Prelu, alpha=alpha02[:, 0:1])
                                    exwide = gw.tile([128, HEADS * nc_t], BF,
                                                     tag="exw")
                                    nc.scalar.activation(exwide[:], ev[:],
                                                         AF.Exp)
                                    for k in range(nc_t):
                                        rhs = gw.tile([128, F_GAT], BF,
                                                      tag="rhs")
                                        xg_b = _bc(xg_t[:, F_IN * k:
                                                        F_IN * k + 1],
                                                   [[0, HEADS], [1, F_HEAD]])
                                        ex_b = _bc(exwide[:, 10 * k:
                                                          10 * k + 1],
                                                   [[1, HEADS], [0, F_HEAD]])
                                        veng = (nc.gpsimd if k % 3 == 2
                                                else nc.vector)
                                        veng.tensor_tensor(
                                            rhs[:].rearrange(
                                                "p (h f) -> p h f", h=HEADS),
                                            xg_b, ex_b, OP.mult)
                                        mask = masks_t[:, 128 * k:128 * (k + 1)]
                                        st, sp = (k == 0), (k == nc_t - 1)
                                        nc.tensor.matmul(psum_g[:, 0:512],
                                                         mask, rhs[:, 0:512],
                                                         start=st, stop=sp)
                                        nc.tensor.matmul(psum_g[:, 512:660],
                                                         mask, rhs[:, 512:660],
                                                         start=st, stop=sp)
                                        nc.tensor.matmul(
                                            psum_e[:, HEADS * nc_t:
                                                   HEADS * nc_t + HEADS],
                                            mask,
                                            exwide[:, 10 * k:10 * (k + 1)],
                                            start=st, stop=sp)
                                else:
                                    for k in range(nc_t):
                                        pse_t = pse.tile([128, HEADS], F32,
                                                         tag=f"pse{k % 2}",
                                                         name=f"pse{k % 2}")
                                        ps_e = pse_t[:]
                                        nc.tensor.matmul(
                                            ps_e,
                                            xgT_t[:, 128 * k:128 * (k + 1)],
                                            was_sb[:], start=True, stop=False)
                                        nc.tensor.matmul(
                                            ps_e,
                                            masksT_t[:, 128 * k:128 * (k + 1)],
                                            ad_tile, start=False, stop=True)
                                        ev = gw.tile([128, HEADS], F32,
                                                     tag="ev")
                                        nc.scalar.activation(
                                            ev[:], ps_e, AF.Prelu,
                                            alpha=alpha02[:, 0:1])
                                        rhs = gw.tile([128, 670], BF,
                                                      tag="rhs")
                                        nc.scalar.activation(
                                            rhs[:, 660:670], ev[:], AF.Exp)
                                        xg_b = _bc(xg_t[:, F_IN * k:
                                                        F_IN * k + 1],
                                                   [[0, HEADS], [1, F_HEAD]])
                                        ex_b = _bc(rhs[:, 660:661],
                                                   [[1, HEADS], [0, F_HEAD]])
                                        veng = (nc.gpsimd if k % 4 == 3
                                                else nc.vector)
                                        veng.tensor_tensor(
                                            rhs[:, 0:F_GAT].rearrange(
                                                "p (h f) -> p h f", h=HEADS),
                                            xg_b, ex_b, OP.mult)
                                        mask = masks_t[:, 128 * k:128 * (k + 1)]
                                        st, sp = (k == 0), (k == nc_t - 1)
                                        nc.tensor.matmul(psum_g[:, 0:512],
                                                         mask, rhs[:, 0:512],
                                                         start=st, stop=sp)
                                        nc.tensor.matmul(psum_g[:, 512:670],
                                                         mask, rhs[:, 512:670],
                                                         start=st, stop=sp)

                                # ---- tile tail: normalize, per-head W, bias+leaky
                                s_sb = gw.tile([128, HEADS], F32, tag="s")
                                if BATCH_E:
                                    s_src = psum_e[:, HEADS * nc_t:
                                                   HEADS * nc_t + HEADS]
                                else:
                                    s_src = psum_g[:, 660:670]
                                nc.vector.tensor_scalar(s_sb[:], s_src,
                                                        1e-6, None, OP.max)
                                rs = gw.tile([128, HEADS], F32, tag="rs")
                                nc.vector.reciprocal(rs[:], s_sb[:])
                                # A_norm: head h at cols [128h, 128h+66),
                                # zero-padded to 128 so transposed blocks are
                                # partition-0 aligned for the per-head matmul.
                                A_norm = gt.tile([128, 1280], BF, tag="anorm")
                                if t < 2:
                                    for h in range(HEADS):
                                        nc.vector.memset(
                                            A_norm[:, 128 * h + 66:
                                                   128 * (h + 1)], 0.0)
                                rs_b = _bc(rs[:, 0:1], [[1, HEADS], [0, F_HEAD]])
                                nc.vector.tensor_tensor(
                                    _bc(A_norm[:, 0:1],
                                        [[128, HEADS], [1, F_HEAD]]),
                                    _bc(psum_g[:, 0:1], [[66, HEADS], [1, F_HEAD]]),
                                    rs_b, OP.mult)
                                psum_h1 = psh.tile([128, 330], F32, tag="psh1")
                                psum_h2 = psh.tile([128, 330], F32, tag="psh2")
                                for h in range(HEADS):
                                    ph = psum_h1 if h < 5 else psum_h2
                                    o = 66 * h - (0 if h < 5 else 330)
                                    tp = pst.tile([96, 128], BF, tag="tp")
                                    nc.tensor.transpose(
                                        tp[:], A_norm[:, 128 * h:128 * h + 96],
                                        ident_sb[:])
                                    ahT = gw.tile([96, 128], BF, tag="ahT")
                                    if AHT_SCALAR:
                                        nc.scalar.activation(ahT[:], tp[:],
                                                             AF.Copy)
                                    else:
                                        nc.vector.tensor_copy(ahT[:], tp[:])
                                    nc.tensor.matmul(
                                        ph[:, o:o + 66], ahT[0:66, :],
                                        W_heads_sb[:, 66 * h:66 * (h + 1)],
                                        start=True, stop=False)
                                    nc.tensor.matmul(
                                        ph[:, o:o + 66], ones_sb[0:1, 0:128],
                                        b_gat_sb[0:1, 66 * h:66 * (h + 1)],
                                        start=False, stop=True)
                                h_tile = gt.tile([128, HW_H], BF, tag="htile")
                                if t < 2:
                                    nc.vector.memset(h_tile[:, 660:HW_H], 0.0)
                                for half, phs in ((0, psum_h1), (1, psum_h2)):
                                    nc.scalar.activation(
                                        h_tile[:, 330 * half:330 * (half + 1)],
                                        phs[:, 0:330], AF.Lrelu)
                                qi = q_of_tile[t]
                                tt = t - sum(QT[:qi])
                                nc.gpsimd.dma_start(
                                    h_myQ[qi][128 * tt:128 * (tt + 1), :],
                                    h_tile[:])
                                if (SPLIT_AG and "G" in phases
                                        and t == q_last_tile[qi]
                                        and qi < len(QT) - 1):
                                    nc.gpsimd.collective_compute(
                                        "AllGather", OP.bypass,
                                        replica_groups=[core_ids],
                                        ins=[h_myQ[qi].opt()],
                                        outs=[h_full[q_base[qi]:
                                                     q_base[qi] + NCORE * 128
                                                     * QT[qi], :]])

                    # ---------------- AllGather (remaining quarters) --------
                    if "G" in phases:
                        qs = ([len(QT) - 1] if SPLIT_AG
                              else list(range(len(QT))))
                        for qi in qs:
                            nc.gpsimd.collective_compute(
                                "AllGather", OP.bypass,
                                replica_groups=[core_ids],
                                ins=[h_myQ[qi].opt()],
                                outs=[h_full[q_base[qi]:
                                             q_base[qi] + NCORE * 128
                                             * QT[qi], :]])

                    if dump_h:
                        nc.gpsimd.dma_start(hdump_d[:], h_full[:])

                    # ---------------- phase C: GCN + feature-major MLP ----------
                    if "C" in phases:
                        with tc.tile_pool(name="hg_pool", bufs=3) as hgp, \
                             tc.tile_pool(name="gcn_work", bufs=4) as gcw, \
                             tc.tile_pool(name="grp", bufs=2) as grp, \
                             tc.tile_pool(name="psumC", bufs=2,
                                          space=bass.MemorySpace.PSUM) as psc, \
                             tc.tile_pool(name="psumM", bufs=2,
                                          space=bass.MemorySpace.PSUM) as psm:

                            kws = [128] * 5 + [32]
                            kws1 = [128] * 10 + [40]
                            kws2 = [128] * 7 + [104]
                            groups = [(0, 4), (4, 4), (8, 2)]

                            def dense_stack(g0, gn, aggT):
                                nw = gn * 128
                                gT = grp.tile([128, 11 * 512], BF, tag="gT")
                                for mc in range(11):
                                    mw = 128 if mc < 10 else 40
                                    ps = psm.tile([128, 512], F32, tag="psm")
                                    for kt in range(6):
                                        nc.tensor.matmul(
                                            ps[0:mw, 0:nw],
                                            W_gcn_sb[0:kws[kt],
                                                     GCN_OUT * kt + 128 * mc:
                                                     GCN_OUT * kt + 128 * mc + mw],
                                            aggT[0:kws[kt],
                                                 512 * kt:512 * kt + nw],
                                            start=(kt == 0), stop=(kt == 5))
                                    nc.scalar.activation(
                                        gT[0:mw, 512 * mc:512 * mc + nw],
                                        ps[0:mw, 0:nw], AF.Lrelu,
                                        bias=b_gcn_sb[0:mw, mc:mc + 1])

                                z1T = grp.tile([128, 8 * 512], BF, tag="z1T")
                                for mc in range(8):
                                    mw = 128 if mc < 7 else 104
                                    ps = psm.tile([128, 512], F32, tag="psm")
                                    for kt in range(11):
                                        nc.tensor.matmul(
                                            ps[0:mw, 0:nw],
                                            W_g1_sb[0:kws1[kt],
                                                    1000 * kt + 128 * mc:
                                                    1000 * kt + 128 * mc + mw],
                                            gT[0:kws1[kt],
                                               512 * kt:512 * kt + nw],
                                            start=(kt == 0), stop=(kt == 10))
                                    nc.scalar.activation(
                                        z1T[0:mw, 512 * mc:512 * mc + nw],
                                        ps[0:mw, 0:nw], AF.Lrelu,
                                        bias=b_g1_sb[0:mw, mc:mc + 1])

                                ps2 = psm.tile([128, 512], F32, tag="psm")
                                for kt in range(8):
                                    nc.tensor.matmul(
                                        ps2[0:64, 0:nw],
                                        W_g2_sb[0:kws2[kt], 64 * kt:64 * kt + 64],
                                        z1T[0:kws2[kt], 512 * kt:512 * kt + nw],
                                        start=(kt == 0), stop=(kt == 7))
                                z2T = gcw.tile([64, 512], BF, tag="z2T")
                                nc.scalar.activation(z2T[:, 0:nw], ps2[0:64, 0:nw],
                                                     AF.Lrelu,
                                                     bias=b_tail_sb[0:64, 0:1])

                                ps3 = psm.tile([128, 512], F32, tag="psm")
                                nc.tensor.matmul(ps3[0:32, 0:nw], W_fc1_sb[:],
                                                 z2T[0:64, 0:nw],
                                                 start=True, stop=True)
                                z3T = gcw.tile([32, 512], BF, tag="z3T")
                                nc.scalar.activation(z3T[:, 0:nw], ps3[0:32, 0:nw],
                                                     AF.Lrelu,
                                                     bias=b_tail_sb[0:32, 1:2])

                                ps4 = psm.tile([128, 512], F32, tag="psm")
                                nc.tensor.matmul(ps4[0:16, 0:nw], W_fc2_sb[:],
                                                 z3T[0:32, 0:nw],
                                                 start=True, stop=True)
                                z4T = gcw.tile([16, 512], BF, tag="z4T")
                                nc.scalar.activation(z4T[:, 0:nw], ps4[0:16, 0:nw],
                                                     AF.Lrelu,
                                                     bias=b_tail_sb[0:16, 2:3])

                                ps5 = psm.tile([128, 512], F32, tag="psm")
                                nc.tensor.matmul(ps5[0:1, 0:nw], W_out_sb[:],
                                                 z4T[0:16, 0:nw],
                                                 start=True, stop=True)
                                outT = gcw.tile([1, 512], F32, tag="outT")
                                nc.scalar.activation(outT[0:1, 0:nw],
                                                     ps5[0:1, 0:nw], AF.Identity,
                                                     bias=b_tail_sb[0:1, 3:4])
                                nc.sync.dma_start(
                                    y_d[0:1, 128 * g0:128 * g0 + nw],
                                    outT[0:1, 0:nw])

                            for g0, gn in groups:
                                aggT = grp.tile([128, 6 * 512], BF, tag="aggT")
                                for j in range(gn):
                                    t = g0 + j
                                    # transposed aggregation: psum_bT[f, dst]
                                    # = sum_e hg[e, f] * wmask[e, dst]
                                    psum_bT = psc.tile([128, 6 * 128], F32,
                                                       tag="psa")
                                    hg = hgp.tile([128, nc_t * HW_H], BF,
                                                  tag="hg")
                                    h1 = (nc_t + 1) // 2
                                    o = t * nc_t * 8
                                    for hi, (ka, kb) in enumerate(
                                            ((0, h1), (h1, nc_t))):
                                        nidx = (kb - ka) * 128
                                        nc.gpsimd.dma_gather(
                                            hg[:, ka * HW_H:kb * HW_H]
                                            .rearrange("p (k w) -> p k w",
                                                       w=HW_H),
                                            h_full[:],
                                            sidx_sb[:, o + ka * 8:o + kb * 8],
                                            num_idxs=nidx,
                                            num_idxs_reg=nidx,
                                            elem_size=HW_H,
                                            queue_num=(2 * t + hi) % NSWQ)
                                    # fb-major: each zero-region group is
                                    # fully start->stop before the next opens
                                    for fb in range(6):
                                        for k in range(nc_t):
                                            c = t * nc_t + k
                                            wmask = wmasks_sb[:, 128 * c:
                                                              128 * (c + 1)]
                                            st, sp = (k == 0), (k == nc_t - 1)
                                            nc.tensor.matmul(
                                                psum_bT[:, 128 * fb:
                                                        128 * (fb + 1)],
                                                hg[:, HW_H * k + 128 * fb:
                                                   HW_H * k + 128 * (fb + 1)],
                                                wmask, start=st, stop=sp)
                                    for b in range(6):
                                        w = kws[b]
                                        nc.vector.tensor_copy(
                                            aggT[0:w, 512 * b + 128 * j:
                                                 512 * b + 128 * (j + 1)],
                                            psum_bT[0:w, 128 * b:128 * (b + 1)])
                                dense_stack(g0, gn, aggT)

    nc.compile()
    return nc


# ---------------------------------------------------------------- entry point

def kernel(x, edge_index, W_gat, att_src, att_dst, b_gat, W_gcn, b_gcn,
           W_g1, b_g1, W_g2, b_g2, W_fc1, b_fc1, W_fc2, b_fc2, W_out, b_out,
           _want_trace=False):
    x = np.asarray(x, np.float32)
    edge_index = np.asarray(edge_index)
    prep = _prep(x, edge_index)
    wts = _prep_weights(W_gat, att_src, att_dst, b_gat, W_gcn, b_gcn,
                        W_g1, b_g1, W_g2, b_g2, W_fc1, b_fc1, W_fc2, b_fc2,
                        W_out, b_out)

    nc_t = prep["nc_t"]
    if nc_t not in _CACHE:
        _CACHE[nc_t] = _build(nc_t)
    nc = _CACHE[nc_t]

    in_maps = _in_maps(prep, wts)
    res = run_bass_kernel_spmd(nc, in_maps, list(range(NCORE)),
                               trace=_want_trace)
    y_all = np.concatenate([np.asarray(res.results[c]["y"]).reshape(-1)
                            for c in range(NCORE)])
    out = y_all[prep["slot"]].astype(np.float32).reshape(N, 1)
    if _want_trace:
        return out, res
    return out


def _in_maps(prep, wts):
    shared = dict(
        w_ad=wts["w_ad"], w_as_bf=wts["w_as_bf"], W_heads=wts["W_heads"],
        b_gat_row=wts["b_gat_row"], ones_row=wts["ones_row"],
        ident=wts["ident"],
        W_gcn_p=wts["W_gcn_p"], W_g1_p=wts["W_g1_p"], W_g2_p=wts["W_g2_p"],
        b_gcn_cols=wts["b_gcn_cols"], b_g1_cols=wts["b_g1_cols"],
        W_fc1_p=wts["W_fc1_p"], W_fc2_p=wts["W_fc2_p"],
        W_out_p=wts["W_out_p"], b_tail=wts["b_tail"])
    in_maps = []
    for c in range(NCORE):
        m = dict(shared)
        for k2 in ["xg", "xgT", "masks", "masksT", "wmasks", "idx16", "xT_my"]:
            m[k2] = prep[k2][c]
        in_maps.append(m)
    return in_maps


if __name__ == "__main__":
    sys.path.insert(0, os.path.dirname(os.path.abspath(__file__)))
    import reference
    inputs = reference.setup_inputs()
    inputs = {k: np.asarray(v) for k, v in inputs.items()}
    expected = np.asarray(reference.reference(**inputs))
    got = kernel(**inputs)
    err = np.linalg.norm(got - expected) / np.linalg.norm(expected)
    print("Relative error:", err)

